# revision 19
# baseline (speedup 1.0000x reference)
"""Causal self-attention with RoPE on 8 TRN2 NeuronCores.

Sharding: tensor-parallel over heads (H=8 -> 1 head per core).

v2.4 design — single merged loop, iterations [tb0+tb1], [ib0], [ib1+tb2],
..., [ib6+tb7], [ib7]:
  - PE warm-up matmuls at t=0 (HAM runs the PE at 1.2 GHz until it sees a
    ~3.4us sustained-busy window; the warm-up ramps it to 2.4 GHz while the
    first input DMAs land).
  - qk projection: one matmul set per t-block; W rows pre-permuted to
    [q_even(32), q_odd(32), k_even(32), k_odd(32)] so the RoPE pair-swap
    becomes 32-row-block cross-partition DVE muls against a f32 PSUM source
    (cross-partition-base ops are legal when one operand is in PSUM).
  - S^T blocks (j,i) via row-paired concurrent 64-contraction matmuls into
    one (128,2,TB) 2-bank PSUM tile per chunk-pair.
  - exp(S^T/8) on ACT: ONE activation per chunk-pair (up to 1024 cols) in
    bf16. Causal diag masking via Pool memset + affine_select.
  - PV via [v | ones]-stationary matmuls per chunk; row 64 of the PSUM
    accumulator is the softmax denominator.
  - per i-block, yt (65,TB) is copied to SBUF bf16 and DMA'd out; the host
    normalizes, applies c_proj, and sums heads (the tensor-parallel gather).
"""
import sys

sys.path.insert(0, "/opt/trn_rl_repo")

import numpy as np
import ml_dtypes

import concourse.bass as bass
import concourse.mybir as mybir
import concourse.tile as tile
from concourse.bass_utils import run_bass_kernel_spmd

B, T, C, H = 1, 4096, 512, 8
HS = C // H  # 64
NCORES = 8
TB = 512           # t-block width for projections / i-block width
NTB = T // TB      # 8
JC = 128           # j-chunk width
NJC = T // JC      # 32

_ctr = [0]


def _legalize_waits(nc):
    """This walrus build accepts at most one sem-wait command per hw
    instruction; move extra waits onto same-engine NoOps inserted before."""
    for f in nc.m.functions:
        for bb in f.blocks:
            insts = bb.instructions
            out = []
            for inst in insts:
                si = inst.sync_info
                if si is not None and len(si.on_wait) > 1:
                    waits = list(si.on_wait)
                    for w in waits[:-1]:
                        _ctr[0] += 1
                        nop = mybir.InstNoOp(name=f"I-waitsplit-{_ctr[0]}")
                        nop.engine = inst.engine
                        nop.sync_info = mybir.SyncInfo(on_wait=[w], on_update=[])
                        out.append(nop)
                    inst.sync_info = mybir.SyncInfo(
                        on_wait=[waits[-1]], on_update=list(si.on_update)
                    )
                out.append(inst)
            insts[:] = out
    return nc


def _build_nc():
    nc = bass.Bass()
    f32 = mybir.dt.float32
    bf16 = mybir.dt.bfloat16
    Exp = mybir.ActivationFunctionType.Exp

    xt_in = nc.declare_dram_parameter("xt", [C, T], bf16, isOutput=False)
    wqk_in = nc.declare_dram_parameter("wqk", [C, 128], bf16, isOutput=False)
    wv_in = nc.declare_dram_parameter("wv", [C, HS], bf16, isOutput=False)
    cc_in = nc.declare_dram_parameter("cc", [128, T], f32, isOutput=False)
    ss_in = nc.declare_dram_parameter("ss", [128, T], f32, isOutput=False)
    yu_out = nc.declare_dram_parameter("yu", [HS + 1, T], bf16, isOutput=True)

    with tile.TileContext(nc) as tc:
        with (
            tc.tile_pool(name="big", bufs=1) as big,
            tc.tile_pool(name="ropet", bufs=3) as ropet,
            tc.tile_pool(name="ptp", bufs=6) as ptp,
            tc.tile_pool(name="ytsb", bufs=2) as ytsb,
            tc.tile_pool(name="qkp", bufs=2, space="PSUM") as qkp,
            tc.tile_pool(name="stp", bufs=2, space="PSUM") as stp,
            tc.tile_pool(name="yvo", bufs=2, space="PSUM") as yvo,
        ):
            # ---- resident inputs: ordered for the fastest possible start:
            # xt0 + wqk feed qk(tb0); cc0/ss0 feed its rope; wv feeds v. ----
            wqk_sb = big.tile([128, 4, 128], bf16)
            wv_sb = big.tile([128, 4, HS], bf16)
            xt_sb = big.tile([128, 4, T], bf16)
            cc_sb = big.tile([128, T], f32)
            ss_sb = big.tile([128, T], f32)
            _xt_r = xt_in.ap().rearrange("(n p) t -> p n t", p=128)

            def emit_inputs(tb):
                t0 = tb * TB
                nc.sync.dma_start(out=xt_sb[:, :, t0:t0 + TB],
                                  in_=_xt_r[:, :, t0:t0 + TB])
                nc.sync.dma_start(out=cc_sb[:, t0:t0 + TB],
                                  in_=cc_in.ap()[:, t0:t0 + TB])
                nc.sync.dma_start(out=ss_sb[:, t0:t0 + TB],
                                  in_=ss_in.ap()[:, t0:t0 + TB])

            nc.sync.dma_start(out=wqk_sb, in_=wqk_in.ap().rearrange("(n p) m -> p n m", p=128))
            nc.sync.dma_start(out=wv_sb, in_=wv_in.ap().rearrange("(n p) m -> p n m", p=128))
            emit_inputs(0)
            emit_inputs(1)

            scratch = big.tile([64, TB], bf16)
            nc.vector.memset(scratch, 0.5)
            qkr = big.tile([128, T], bf16)    # rows 0:64 q_rot^T, 64:128 k_rot^T
            krqr = big.tile([128, T], bf16)   # swapped 64-row blocks
            v_ones = big.tile([128, NJC, HS + 1], bf16)
            nc.vector.memset(v_ones[:, :, HS], 1.0)

            pend_pv = []    # (pt, m, ve, vo, nj2) awaiting PV matmul
            state = {"yt": None}
            done_v = set()

            def emit_qk(tb):
                tc0 = tb * TB
                qk_ps = qkp.tile([128, TB], f32, tag="qk")
                for cn in range(4):
                    nc.tensor.matmul(qk_ps, wqk_sb[:, cn, :],
                                     xt_sb[:, cn, tc0:tc0 + TB],
                                     start=(cn == 0), stop=(cn == 3))
                sl = slice(tc0, tc0 + TB)
                t2 = ropet.tile([128, TB], f32, tag="rt")
                nc.vector.tensor_mul(t2, qk_ps, cc_sb[:, sl])
                t1 = ropet.tile([128, TB], f32, tag="rt")
                nc.vector.tensor_mul(t1[0:32], qk_ps[32:64], ss_sb[0:32, sl])
                nc.vector.tensor_mul(t1[32:64], qk_ps[0:32], ss_sb[32:64, sl])
                nc.vector.tensor_mul(t1[64:96], qk_ps[96:128], ss_sb[64:96, sl])
                nc.vector.tensor_mul(t1[96:128], qk_ps[64:96], ss_sb[96:128, sl])
                nc.vector.tensor_add(qkr[:, sl], t2, t1)
                nc.sync.dma_start(out=krqr[0:64, sl], in_=qkr[64:128, sl])
                nc.sync.dma_start(out=krqr[64:128, sl], in_=qkr[0:64, sl])

            def emit_v(tb):
                tc0 = tb * TB
                v_ps = yvo.tile([128, 4, HS], f32, tag="yvo")
                for t4 in range(4):
                    p0 = tc0 + t4 * 128
                    for cn in range(4):
                        nc.tensor.matmul(v_ps[:, t4, :],
                                         xt_sb[:, cn, p0:p0 + 128],
                                         wv_sb[:, cn, :],
                                         start=(cn == 0), stop=(cn == 3),
                                         skip_group_check=True)
                nc.vector.tensor_copy(v_ones[:, 4 * tb:4 * tb + 4, 0:HS], v_ps)

            def flush_pv(n=0):
                while len(pend_pv) > n:
                    pt_, m_, ve_, vo_, nj2_ = pend_pv.pop(0)
                    if m_ == 0:
                        state["yt"] = yvo.tile([HS + 1, TB], f32, tag="yvo",
                                               name="yt_ps")
                    for s_, v0_ in ((0, ve_), (1, vo_)):
                        j_ = 2 * m_ + s_
                        nc.tensor.matmul(state["yt"][:, v0_:TB],
                                         v_ones[:, j_, :],
                                         pt_[:, s_, v0_:TB],
                                         start=(j_ == 0),
                                         stop=(j_ == 2 * nj2_ - 1),
                                         skip_group_check=True)

            def emit_pair(ib, m):
                """S^T pair + merged exp + mask for chunks (2m, 2m+1)."""
                i0 = ib * TB
                nj2 = 2 * ib + 2
                j_e, j_o = 2 * m, 2 * m + 1
                ve = max(0, j_e * JC - i0)
                vo = max(0, j_o * JC - i0)
                st = stp.tile([128, 2, TB], f32, tag="st")
                nc.tensor.matmul(st[:, 0, ve:TB],
                                 krqr[0:64, j_e * JC:(j_e + 1) * JC],
                                 qkr[0:64, i0 + ve:i0 + TB],
                                 tile_position=(0, 0), skip_group_check=True)
                nc.tensor.matmul(st[:, 1, vo:TB],
                                 qkr[64:128, j_o * JC:(j_o + 1) * JC],
                                 krqr[64:128, i0 + vo:i0 + TB],
                                 tile_position=(64, 0), skip_group_check=True)
                pt = ptp.tile([128, 2, TB], mybir.dt.bfloat16, tag="pt")
                stf = st.rearrange("p two f -> p (two f)")
                ptf = pt.rearrange("p two f -> p (two f)")
                nc.scalar.activation(ptf[:, ve:2 * TB], stf[:, ve:2 * TB],
                                     Exp, scale=0.125)
                if vo > ve:  # zero masked-out prefix of the odd chunk
                    nc.gpsimd.memset(pt[:, 1, ve:vo], 0.0)
                for s, j in ((0, j_e), (1, j_o)):
                    if j * JC + JC - 1 > i0:  # diagonal band
                        b0 = max(0, j * JC - i0)
                        b1 = min(TB, b0 + JC)
                        nc.gpsimd.affine_select(
                            out=pt[:, s, b0:b1], in_=pt[:, s, b0:b1],
                            compare_op=mybir.AluOpType.is_ge,
                            fill=0.0, base=i0 + b0 - j * JC,
                            pattern=[[1, b1 - b0]], channel_multiplier=-1)
                pend_pv.append((pt, m, ve, vo, nj2))

            def emit_ib_tail(ib):
                i0 = ib * TB
                flush_pv(0)
                yt_sb = ytsb.tile([HS + 1, TB], mybir.dt.bfloat16, tag="yts")
                nc.vector.tensor_copy(yt_sb, state["yt"])
                nc.sync.dma_start(out=yu_out.ap()[:, i0:i0 + TB], in_=yt_sb)

            # ---- PE warm-up: dummy matmuls, no input deps ----
            warm_ps = stp.tile([128, 2, TB], f32, tag="st")
            for _w in range(16):
                nc.tensor.matmul(warm_ps[:, 0, :], scratch[:, 0:128], scratch,
                                 skip_group_check=True)

            # ---- iteration schedule ----
            emit_qk(0)
            emit_v(0)
            if NTB > 1:
                emit_qk(1)
                emit_v(1)
            if NTB > 2:
                emit_inputs(2)
            for ib in range(NTB):
                npair = 2 * ib + 2
                for m in range(npair):
                    emit_pair(ib, m)
                    if m == 1 and ib >= 1 and ib + 1 < NTB:
                        emit_qk(ib + 1)
                    flush_pv(2)
                    if m == 2 and ib + 1 < NTB and ib + 1 not in done_v:
                        emit_v(ib + 1)  # after yt(ib) alloc in flush_pv
                    if m == 3 and ib + 2 < NTB:
                        emit_inputs(ib + 2)
                emit_ib_tail(ib)

    _legalize_waits(nc)
    return nc


_cached = {}


def _get_nc():
    if "nc" not in _cached:
        _cached["nc"] = _build_nc()
    return _cached["nc"]


def _prep_inputs(x, rope, W_attn, W_proj):
    bf16 = ml_dtypes.bfloat16
    xt = np.ascontiguousarray(x[0].T).astype(bf16)          # (C, T)
    cos = np.asarray(rope[..., 0], dtype=np.float32)        # (T, HS//2)
    sin = np.asarray(rope[..., 1], dtype=np.float32)
    cosT = np.ascontiguousarray(cos.T)                      # (32, T)
    sinT = np.ascontiguousarray(sin.T)
    cc = np.ascontiguousarray(
        np.concatenate([cosT, cosT, cosT, cosT], axis=0))   # (128, T)
    ss = np.ascontiguousarray(
        np.concatenate([-sinT, sinT, -sinT, sinT], axis=0))

    Wa = np.asarray(W_attn, dtype=np.float32)
    # HS permutation: even components first, odd second
    perm = np.concatenate([np.arange(0, HS, 2), np.arange(1, HS, 2)])

    in_maps = []
    for h in range(NCORES):
        Wq = Wa[h * HS:(h + 1) * HS][perm]                  # (HS, C)
        Wk = Wa[C + h * HS:C + (h + 1) * HS][perm]
        Wv = Wa[2 * C + h * HS:2 * C + (h + 1) * HS]
        wqk = np.concatenate([Wq.T, Wk.T], axis=1).astype(bf16)  # (C, 128)
        wv = np.ascontiguousarray(Wv.T).astype(bf16)             # (C, HS)
        in_maps.append({
            "xt": xt, "wqk": np.ascontiguousarray(wqk),
            "wv": wv, "cc": cc, "ss": ss,
        })
    return in_maps


def run_cores(x, rope, W_attn, W_proj, trace=False):
    nc = _get_nc()
    in_maps = _prep_inputs(x, rope, W_attn, W_proj)
    res = run_bass_kernel_spmd(nc, in_maps, list(range(NCORES)), trace=trace)
    return res


def kernel(x, rope, mask, W_attn, W_proj):
    res = run_cores(x, rope, W_attn, W_proj, trace=False)
    Wp = np.asarray(W_proj, dtype=np.float32)
    out = np.zeros((T, C), dtype=np.float32)
    for h in range(NCORES):
        yu = np.asarray(res.results[h]["yu"], dtype=np.float32)  # (65, T)
        y = (yu[0:HS] / yu[HS:HS + 1]).T                         # (T, 64)
        out += y @ Wp[:, h * HS:(h + 1) * HS].T
    return out.reshape(B, T, C).astype(np.float32)
